# revision 71
# baseline (speedup 1.0000x reference)
"""Multi-head attention (B=2, S=2048, D=1024, H=16, causal) on 8 Trainium2
NeuronCores via Bass/Tile.

Sharding: core c -> batch c//4, heads [4*(c%4), 4*(c%4)+4)  (DP over batch x
TP over heads).  QKV weights column-parallel, O row-parallel; the 4 partial
[S, D] outputs per batch are summed on the host (gather step), bias bo added
there too.

v2 dataflow (bf16 matmuls, fp32 PSUM accumulation), per core:
  - single sync-engine DMA queue in needed-by order: wk, bk, xk tiles woven
    with wq/cm/..., so the first K-proj matmul isn't stuck behind weights.
  - K proj -> kt [128 (2 heads x 64dk), S] x2 pairs; Q proj -> qt same.
  - scores for qc=0 are woven into the Q/V projection phase (Scalar exp
    starts ~25us in instead of ~55us).
  - scores computed transposed per kc block [128 k, 2, 512 q] covering BOTH
    heads of a pair; the two matmuls use partition rows 0:64 / 64:128 so the
    PE row-group tiling runs them concurrently.  ONE exp ACT covers both
    heads.  Diagonal blocks restrict to the causally valid q range
    (cols >= 128*j) for scores, exp, and attnV; the 128-wide diagonal strip
    gets the triangular keep-mask on DVE.
  - attnV: po[65, 512] per head; row 64 accumulates the softmax denominator
    via the ones column in V'.  kc loop is software-pipelined (attnV lags
    scores by 2) so the PE never waits on the exp chain.
  - normalize: DVE reciprocal of the denominator row -> rcpg[4, 512] bf16;
    one PE matmul (sel2) broadcasts a PAIR's two reciprocal rows to
    [128, 512]; gpsimd multiplies into outT2[pair] [128 (2 heads), S] bf16.
  - O proj: out[q, D] = sum_p outT2[p].T @ wo2[p] -- 2 accumulating matmuls
    per [128, 512] tile (full 128-deep contraction).
  - bc/normalize/O-proj work is queued as "fillers" popped one per kc
    iteration so the PE stream has no micro-idles (HAM stays at K=8/8).
"""

import os
import sys
import types
from collections import deque

import numpy as np

B, S, D, H = 2, 2048, 1024, 16
DK = D // H  # 64
N_CORES = 8
HPC = 4  # heads per core
SCALE = 1.0 / np.sqrt(np.float32(DK))  # folded into Wq/bq on host

QC = 512  # query block (free dim of scores matmuls)
KC = 128  # key block (partition dim of transposed scores)
NQC = S // QC  # 4
LAG = 5  # attnV trails scores by LAG kc-iterations


def _install_ntff_hook():
    """The image's antenv lacks axon_hooks; register the NTFF profile hook
    ourselves so run_bass_kernel_spmd(trace=True) works."""
    if "antenv.axon_hooks" in sys.modules:
        return
    try:
        mod = types.ModuleType("antenv.axon_hooks")
        state = {"hook": None}
        mod.set_axon_ntff_profile_hook = lambda h: state.__setitem__("hook", h)
        mod.get_axon_ntff_profile_hook = lambda: state["hook"]
        sys.modules["antenv.axon_hooks"] = mod
        from trn_agent_boot.trn_boot import _ntff_profile_via_ctypes

        mod.set_axon_ntff_profile_hook(
            _ntff_profile_via_ctypes("/opt/axon/libaxon_pjrt.so")
        )
    except Exception:
        sys.modules.pop("antenv.axon_hooks", None)


def _split_multi_waits(nc):
    """This walrus build accepts at most ONE sem wait per instruction; Tile
    packs several.  Split extras into preceding single-wait NOPs on the same
    engine (equivalent semantics: the engine blocks on them in order)."""
    import bass_rust

    cnt = 0
    for bbw in nc.main_func.blocks:
        bb = bbw.bb if hasattr(bbw, "bb") else bbw
        out = []
        changed = False
        for ins in bb.instructions:
            si = ins.sync_info
            if si is not None and len(si.on_wait) > 1:
                changed = True
                waits = list(si.on_wait)
                for w in waits[:-1]:
                    cnt += 1
                    nop = bass_rust.InstNoOp(name=f"I-wsp{cnt}", ins=[], outs=[])
                    nop.engine = ins.engine
                    nop.sync_info = bass_rust.SyncInfo(on_wait=[w], on_update=[])
                    out.append(nop)
                si.on_wait = [waits[-1]]
                ins.sync_info = si
            out.append(ins)
        if changed:
            bb.instructions = out
    return cnt


def _build_nc(split=True):
    from contextlib import ExitStack

    import concourse.bass as bass
    import concourse.tile as tile
    from concourse import mybir

    bf16 = mybir.dt.bfloat16
    f32 = mybir.dt.float32

    nc = bass.Bass()
    xqT = nc.declare_dram_parameter("xqT", [D, S], bf16, isOutput=False)
    xkT = nc.declare_dram_parameter("xkT", [D, S], bf16, isOutput=False)
    xvT = nc.declare_dram_parameter("xvT", [D, S], bf16, isOutput=False)
    wq = nc.declare_dram_parameter("wq", [128, 8 * 256], bf16, isOutput=False)
    wk = nc.declare_dram_parameter("wk", [128, 8 * 256], bf16, isOutput=False)
    wv = nc.declare_dram_parameter("wv", [128, 8 * 260], bf16, isOutput=False)
    wo2 = nc.declare_dram_parameter("wo2", [128, 2048], bf16, isOutput=False)
    bq = nc.declare_dram_parameter("bq", [128, 2], f32, isOutput=False)
    bk = nc.declare_dram_parameter("bk", [128, 2], f32, isOutput=False)
    bvp = nc.declare_dram_parameter("bvp", [1, 260], f32, isOutput=False)
    cm2 = nc.declare_dram_parameter("cm2", [128, 256], bf16, isOutput=False)
    sel2 = nc.declare_dram_parameter("sel2", [128, 256], bf16, isOutput=False)
    outp = nc.declare_dram_parameter("outp", [S, D], bf16, isOutput=True)

    with tile.TileContext(nc) as tc, ExitStack() as ctx:
        consts = ctx.enter_context(tc.tile_pool(name="consts", bufs=1))
        xs = ctx.enter_context(tc.tile_pool(name="xs", bufs=24))
        acts = ctx.enter_context(tc.tile_pool(name="acts", bufs=1))
        exps = ctx.enter_context(tc.tile_pool(name="exps", bufs=26))
        posbp = ctx.enter_context(tc.tile_pool(name="posbp", bufs=4))
        scrp = ctx.enter_context(tc.tile_pool(name="scrp", bufs=4))
        bcsp = ctx.enter_context(tc.tile_pool(name="bcsp", bufs=2))
        osb = ctx.enter_context(tc.tile_pool(name="osb", bufs=4))
        ps_sc = ctx.enter_context(tc.tile_pool(name="ps_sc", bufs=2, space="PSUM"))
        ps1 = ctx.enter_context(tc.tile_pool(name="ps1", bufs=2, space="PSUM"))

        # ---- persistent activation tiles ----
        kt = [acts.tile([128, S], bf16, name=f"kt{m}", tag=f"kt{m}") for m in range(2)]
        qt = [acts.tile([128, S], bf16, name=f"qt{m}", tag=f"qt{m}") for m in range(2)]
        vh_sb = acts.tile([128, 16, 260], bf16, name="vh", tag="vh")
        outT2 = [
            acts.tile([128, S], bf16, name=f"outT2_{p}", tag=f"outT2_{p}")
            for p in range(2)
        ]
        # reciprocal rows live at partitions 32*(2*pair+hh); other partitions
        # stay at the memset value so the sel2 matmul contracts over zeros,
        # never garbage (0*NaN would poison the broadcast).
        rcpg = acts.tile([128, 512], bf16, name="rcpg", tag="rcpg")

        # ---- DMA issue, needed-by order, all on the sync-engine HW queue ----
        # wk comes as two single-writer half tiles so the first projection
        # matmuls only wait on the 256KB they actually read
        wk_h = [consts.tile([128, 1024], bf16, name=f"wk_h{i}") for i in range(2)]
        nc.sync.dma_start(out=wk_h[0][:], in_=wk[:, 0:1024])

        xt = {}  # (input, dc, half) -> [128, 1024] bf16 tile

        def dma_x(src_, key, dc, half):
            t = xs.tile([128, S // 2], bf16, name="xt", tag="xt")
            nc.sync.dma_start(
                out=t[:],
                in_=src_[dc * 128:(dc + 1) * 128, half * 1024:(half + 1) * 1024],
            )
            xt[(key, dc, half)] = t

        dma_x(xkT, "k", 0, 0)
        nc.sync.dma_start(out=wk_h[1][:], in_=wk[:, 1024:2048])
        bk_sb = consts.tile([128, 2], f32, name="bk_sb")
        nc.sync.dma_start(out=bk_sb[:], in_=bk[:])
        for dc in range(1, 8):
            dma_x(xkT, "k", dc, 0)
        wq_sb = consts.tile([128, 8 * 256], bf16, name="wq_sb")
        nc.sync.dma_start(out=wq_sb[:], in_=wq[:])
        bq_sb = consts.tile([128, 2], f32, name="bq_sb")
        nc.sync.dma_start(out=bq_sb[:], in_=bq[:])
        for dc in range(8):
            dma_x(xkT, "k", dc, 1)
        cm2_sb = consts.tile([128, 2, 128], bf16, name="cm2_sb")
        nc.sync.dma_start(out=cm2_sb[:], in_=cm2[:])
        sel2_sb = consts.tile([128, 256], bf16, name="sel2_sb")
        nc.sync.dma_start(out=sel2_sb[:], in_=sel2[:])
        nc.vector.memset(rcpg[:], 0.0)
        ones1 = consts.tile([1, 512], f32, name="ones1")
        nc.vector.memset(ones1[:], 1.0)
        bvp_sb = consts.tile([128, 260], f32, name="bvp_sb")
        nc.sync.dma_start(out=bvp_sb[:], in_=bvp[:].to_broadcast((128, 260)))
        for half in range(2):
            for dc in range(8):
                dma_x(xqT, "q", dc, half)
        wv_sb = consts.tile([128, 8 * 260], bf16, name="wv_sb")
        nc.sync.dma_start(out=wv_sb[:], in_=wv[:])
        for half in range(2):
            for dc in range(8):
                dma_x(xvT, "v", dc, half)
        wo2_sb = consts.tile([128, 2048], bf16, name="wo2_sb")
        nc.sync.dma_start(out=wo2_sb[:], in_=wo2[:])

        # ---- projection helpers ----
        # dc is the OUTER loop so the first matmul only waits on the first
        # input tile (the PE paces with the DMA stream instead of stalling
        # for all 8 chunks).
        def kq_sc(key, wchunk, bsb, dst, sc):
            ps = [ps1.tile([128, 512], f32, name="ps", tag="ps") for _ in range(2)]
            for dc in range(8):
                wt, base = wchunk(dc)
                for m in range(2):
                    nc.tensor.matmul(
                        ps[m][:],
                        lhsT=wt[:, base + m * 128: base + (m + 1) * 128],
                        rhs=xt[(key, dc, sc // 2)][
                            :, (sc % 2) * 512:(sc % 2) * 512 + 512
                        ],
                        start=(dc == 0),
                        stop=(dc == 7),
                    )
            for m in range(2):
                nc.vector.tensor_scalar_add(
                    dst[m][:, sc * 512:(sc + 1) * 512], ps[m][:], bsb[:, m:m + 1]
                )

        def v_stpair(sp):
            sts = (2 * sp, 2 * sp + 1)
            ps = [ps1.tile([128, 512], f32, name="ps", tag="ps") for _ in range(2)]
            for dc in range(8):
                for i, st in enumerate(sts):
                    nc.tensor.matmul(
                        ps[i][:, :260],
                        lhsT=xt[("v", dc, st // 8)][
                            :, (st % 8) * 128:(st % 8 + 1) * 128
                        ],
                        rhs=wv_sb[:, dc * 260:(dc + 1) * 260],
                        start=(dc == 0),
                        stop=(dc == 7),
                    )
            for i, st in enumerate(sts):
                nc.vector.tensor_add(vh_sb[:, st, :], ps[i][:, :260], bvp_sb[:])

        # ---- attention helpers ----
        def emit_scores(qc, kc, pair):
            """scores + exp (+ causal mask) for one kc block, both heads of
            the pair.  Returns (ex tile, lo) for the matching attnV."""
            j = kc - 4 * qc  # diagonal sub-block index, or negative
            lo = 128 * j if j >= 0 else 0
            pss = ps_sc.tile([128, 2, 512], f32, name="pss", tag="pss")
            for hh in range(2):
                hr = slice(64 * hh, 64 * hh + 64)
                nc.tensor.matmul(
                    pss[:, hh, lo:],
                    lhsT=kt[pair][hr, kc * 128:(kc + 1) * 128],
                    rhs=qt[pair][hr, qc * QC + lo:(qc + 1) * QC],
                    start=True,
                    stop=True,
                )
            ex = exps.tile([128, 2, 512], bf16, name="ex", tag="ex")
            nc.scalar.activation(
                ex[:, :, lo:], pss[:, :, lo:], mybir.ActivationFunctionType.Exp
            )
            if j >= 0:
                # triangular keep-mask on the 128-wide diagonal strip
                nc.vector.tensor_mul(
                    ex[:, :, lo:lo + 128], ex[:, :, lo:lo + 128], cm2_sb[:]
                )
            return ex, lo

        def emit_attnv(qc, kc, pair, po, ex, lo):
            last = 4 * qc + 3
            for hh in range(2):
                h = 2 * pair + hh
                nc.tensor.matmul(
                    po[hh][:, lo:],
                    lhsT=vh_sb[:, kc, h * 65:(h + 1) * 65],
                    rhs=ex[:, hh, lo:],
                    start=(kc == 0),
                    stop=(kc == last),
                )

        def pair_end(qc, pair, po):
            """free the po PSUM banks fast: po[hh] frees after one Ln (ACT)
            and one staging copy (DVE), running in parallel.  The Exp half of
            the reciprocal is deferred past the next pair's first exps; bc2
            only needs rcpg ~3 kc later."""
            posb2 = posbp.tile([128, 512], bf16, name="posb2", tag="posb2")
            lgs = []
            with nc.allow_low_precision(reason="attn-out staged bf16"):
                for hh in range(2):
                    lg = scrp.tile([1, 512], f32, name="lg", tag="lg")
                    nc.scalar.activation(
                        lg[:], po[hh][64:65, :], mybir.ActivationFunctionType.Ln
                    )
                    nc.vector.tensor_copy(
                        posb2[64 * hh:64 * hh + 64, :], po[hh][0:64, :]
                    )
                    lgs.append(lg)

            def emit_lns():
                pass

            def emit_rcp():
                with nc.allow_low_precision(reason="rcp bf16, as baseline"):
                    for hh in range(2):
                        r = 32 * (2 * pair + hh)
                        nc.scalar.activation(
                            rcpg[r:r + 1, :],
                            lgs[hh][:],
                            mybir.ActivationFunctionType.Exp,
                            scale=-1.0,
                        )
            return posb2, emit_lns, emit_rcp

        def make_bc_norm(qc, pair, posb2):
            def emit():
                bcps = ps1.tile([128, 512], f32, name="ps", tag="ps")
                nc.tensor.matmul(
                    bcps[:],
                    lhsT=sel2_sb[:, pair * 128:(pair + 1) * 128],
                    rhs=rcpg[:],
                    start=True,
                    stop=True,
                )
                bcs2 = bcsp.tile([128, 512], bf16, name="bcs2", tag="bcs2")
                nc.vector.tensor_copy(bcs2[:], bcps[:])
                nc.gpsimd.tensor_mul(
                    outT2[pair][:, qc * QC:(qc + 1) * QC], posb2[:], bcs2[:]
                )
            return emit

        def make_oproj(qc, g):
            def emit():
                sti, ns = g // 2, g % 2
                st = qc * 4 + sti
                ps = ps1.tile([128, 512], f32, name="ps", tag="ps")
                for p in range(2):
                    nc.tensor.matmul(
                        ps[:],
                        lhsT=outT2[p][:, st * 128:(st + 1) * 128],
                        rhs=wo2_sb[:, p * 1024 + ns * 512: p * 1024 + ns * 512 + 512],
                        start=(p == 0),
                        stop=(p == 1),
                    )
                ot = osb.tile([128, 512], bf16, name="ot", tag="ot")
                with nc.allow_low_precision(reason="partial out bf16, host f64 sum"):
                    nc.vector.tensor_copy(ot[:], ps[:])
                nc.sync.dma_start(
                    out=outp[st * 128:(st + 1) * 128, ns * 512:(ns + 1) * 512],
                    in_=ot[:],
                )
            return emit

        # ---- projections, with qc=0 and qc=1 scores woven in ----
        wk_chunk = lambda dc: (wk_h[dc // 4], (dc % 4) * 256)
        wq_chunk = lambda dc: (wq_sb, dc * 256)
        pre = {}  # (qc, pair, kc) -> (ex, lo)
        for sc in range(4):
            kq_sc("k", wk_chunk, bk_sb, kt, sc)
        kq_sc("q", wq_chunk, bq_sb, qt, 0)
        for sc in range(1, 4):
            for pair in range(2):
                pre[(0, pair, sc - 1)] = emit_scores(0, sc - 1, pair)
            kq_sc("q", wq_chunk, bq_sb, qt, sc)
        for pair in range(2):
            pre[(0, pair, 3)] = emit_scores(0, 3, pair)
        for sp in range(8):
            v_stpair(sp)
            for pair in range(2):
                pre[(1, pair, sp)] = emit_scores(1, sp, pair)

        # ---- attention main loop ----
        # fillers: (cond, emit_fn) -- PE work from earlier qc woven into the
        # kc loop.  cond(kc) delays items whose dependency chain (rcp -> bc2
        # -> norm) is still settling so the PE doesn't stall on them, and
        # gates oproj(qc) on its norm having been EMITTED (program order).
        fillers = deque()
        norm_done = set()  # (qc, pair) whose bc_norm has been emitted
        rcp_done = set()  # (qc, pair) whose deferred Exp has been emitted
        pending_lns = []  # deferred Ln emitters from the previous pair_end
        pending_exps = []  # deferred Exp emitters from the previous pair_end

        def pop_filler(kc):
            for i, (kind, cond, fn) in enumerate(fillers):
                if cond(kc):
                    del fillers[i]
                    fn()
                    return

        for qc in range(NQC):
            for pair in range(2):
                po = [
                    ps1.tile([65, 512], f32, name=f"po{hh}", tag="po")
                    for hh in range(2)
                ]
                nkc = 4 * qc + 4
                if (qc, pair, 0) in pre:
                    for kc in range(nkc):
                        if kc == 0:
                            for fn in pending_lns:
                                fn()
                            pending_lns.clear()
                        ex, lo = pre.pop((qc, pair, kc))
                        emit_attnv(qc, kc, pair, po, ex, lo)
                        if kc == 1:
                            for fn in pending_exps:
                                fn()
                            pending_exps.clear()
                        if kc >= 1:
                            pop_filler(kc)
                else:
                    meta = {}
                    for kc in range(nkc + LAG):
                        if kc < nkc:
                            meta[kc] = emit_scores(qc, kc, pair)
                        if kc == 0:
                            for fn in pending_lns:
                                fn()
                            pending_lns.clear()
                        if kc == 1:
                            for fn in pending_exps:
                                fn()
                            pending_exps.clear()
                        if kc >= LAG:
                            ex, lo = meta.pop(kc - LAG)
                            emit_attnv(qc, kc - LAG, pair, po, ex, lo)
                        # every-other-kc pops leave fillers in reserve for the
                        # pair boundary unless the backlog is deep
                        if kc >= 1 and (kc % 2 == 1 or len(fillers) > 6):
                            pop_filler(kc)
                posb2, emit_lns, emit_rcp = pair_end(qc, pair, po)
                pending_lns.append(emit_lns)

                def rcp_wrap(qc=qc, pair=pair, emit_rcp=emit_rcp):
                    emit_rcp()
                    rcp_done.add((qc, pair))

                pending_exps.append(rcp_wrap)
                inner = make_bc_norm(qc, pair, posb2)

                def bc_wrap(qc=qc, pair=pair, inner=inner):
                    inner()
                    norm_done.add((qc, pair))

                fillers.append(
                    (
                        "bc",
                        lambda kc, qc=qc, pair=pair: kc >= 3
                        and (qc, pair) in rcp_done,
                        bc_wrap,
                    )
                )
                if pair == 1:
                    for g in range(8):
                        fillers.append(
                            (
                                "op",
                                lambda kc, qc=qc: (qc, 1) in norm_done,
                                make_oproj(qc, g),
                            )
                        )
                # boundary: give the PE ready work (kc-independent fillers)
                # while the po-free / exp chains settle
                pop_filler(-1)
                pop_filler(-1)
                pop_filler(-1)
        for fn in pending_lns:
            fn()
        pending_lns.clear()
        for fn in pending_exps:
            fn()
        pending_exps.clear()
        while fillers:
            fillers.popleft()[2]()

    if split:
        _split_multi_waits(nc)
    return nc


_NC_CACHE = None


def _get_nc():
    global _NC_CACHE
    if _NC_CACHE is None:
        _NC_CACHE = _build_nc()
    return _NC_CACHE


def _swizzle_w(wT, block):
    """wT [D, C] -> [128, 8*C] so that out[p, dc*C + j] = wT[dc*128 + p, j]."""
    dcs = wT.shape[0] // 128
    return np.ascontiguousarray(
        wT.reshape(dcs, 128, wT.shape[1]).transpose(1, 0, 2).reshape(128, -1)
    )


def _np_reference(q, k, v, mask, Wq, bq, Wk, bk, Wv, bv, Wo, bo):
    def split_heads(x):
        b, s, _ = x.shape
        return x.reshape(b, s, H, DK).transpose(0, 2, 1, 3)

    qh = split_heads(q @ Wq.T + bq)
    kh = split_heads(k @ Wk.T + bk)
    vh = split_heads(v @ Wv.T + bv)
    scores = np.einsum("bhqd,bhkd->bhqk", qh, kh) / np.sqrt(np.float32(DK))
    scores = np.where(mask, np.float32(-1e9), scores)
    scores = scores - scores.max(axis=-1, keepdims=True)
    e = np.exp(scores)
    attn = e / e.sum(axis=-1, keepdims=True)
    out = np.einsum("bhqk,bhkd->bhqd", attn, vh)
    out = out.transpose(0, 2, 1, 3).reshape(q.shape[0], -1, D)
    return (out @ Wo.T + bo).astype(np.float32)


def kernel(q, k, v, mask, Wq, bq, Wk, bk, Wv, bv, Wo, bo):
    import ml_dtypes

    bf16 = ml_dtypes.bfloat16

    q = np.asarray(q, np.float32)
    k = np.asarray(k, np.float32)
    v = np.asarray(v, np.float32)
    mask = np.asarray(mask, bool)
    Wq = np.asarray(Wq, np.float32)
    bq = np.asarray(bq, np.float32)
    Wk = np.asarray(Wk, np.float32)
    bk = np.asarray(bk, np.float32)
    Wv = np.asarray(Wv, np.float32)
    bv = np.asarray(bv, np.float32)
    Wo = np.asarray(Wo, np.float32)
    bo = np.asarray(bo, np.float32)

    causal = np.triu(np.ones((S, S), dtype=bool), k=1)
    if not np.array_equal(mask.reshape(S, S), causal):
        return _np_reference(q, k, v, mask, Wq, bq, Wk, bk, Wv, bv, Wo, bo)

    _install_ntff_hook()
    from concourse.bass_utils import run_bass_kernel_spmd

    nc = _get_nc()

    # triangular keep-mask for the 128-wide diagonal strip, doubled for the
    # two heads sharing one exp tile: keep iff kk <= qq
    kk = np.arange(128)[:, None]
    qq = np.arange(128)[None, :]
    tri = (kk <= qq).astype(bf16)
    cm2_np = np.concatenate([tri, tri], axis=1)  # [128, 256]

    # sel2[32*(2p+hh), p*128 + m] = 1 for m in the hh half: broadcast-select
    # the pair's two reciprocal rows (at partitions 0/32/64/96) onto 128
    sel2_np = np.zeros((128, 256), np.float32)
    for p in range(2):
        sel2_np[32 * (2 * p), p * 128:p * 128 + 64] = 1.0
        sel2_np[32 * (2 * p + 1), p * 128 + 64:p * 128 + 128] = 1.0
    sel2_np = sel2_np.astype(bf16)

    xT = {}
    for name, x in (("q", q), ("k", k), ("v", v)):
        xT[name] = [np.ascontiguousarray(x[b].T).astype(bf16) for b in range(B)]

    in_maps = []
    for c in range(N_CORES):
        b = c // 4
        g = c % 4
        hs = slice(g * HPC * DK, (g + 1) * HPC * DK)  # 256 rows of W, cols of Wo
        wq_c = _swizzle_w((SCALE * Wq[hs]).T.astype(bf16), 256)
        wk_c = _swizzle_w(Wk[hs].T.astype(bf16), 256)
        # V' with a zero weight column at h*65+64 (ones come via bias row)
        wvT = Wv[hs].T  # [1024, 256]
        wvp = np.zeros((D, 260), np.float32)
        for h in range(HPC):
            wvp[:, h * 65:h * 65 + 64] = wvT[:, h * 64:(h + 1) * 64]
        wv_c = _swizzle_w(wvp.astype(bf16), 260)
        # wo2: pair p columns hold (Wo[:, hs].T)[p*128:(p+1)*128, :]
        woT = np.ascontiguousarray(Wo[:, hs].T)  # [256, 1024]
        wo2_c = np.concatenate([woT[0:128], woT[128:256]], axis=1).astype(bf16)
        bq_c = np.ascontiguousarray(
            (SCALE * bq[hs]).reshape(2, 128).T.astype(np.float32)
        )
        bk_c = np.ascontiguousarray(bk[hs].reshape(2, 128).T.astype(np.float32))
        bvp_c = np.zeros((1, 260), np.float32)
        for h in range(HPC):
            bvp_c[0, h * 65:h * 65 + 64] = bv[hs][h * 64:(h + 1) * 64]
            bvp_c[0, h * 65 + 64] = 1.0
        in_maps.append(
            {
                "xqT": xT["q"][b],
                "xkT": xT["k"][b],
                "xvT": xT["v"][b],
                "wq": wq_c,
                "wk": wk_c,
                "wv": wv_c,
                "wo2": wo2_c,
                "bq": bq_c,
                "bk": bk_c,
                "bvp": bvp_c,
                "cm2": cm2_np,
                "sel2": sel2_np,
            }
        )

    trace = bool(os.environ.get("BASSMHA_TRACE"))
    res = run_bass_kernel_spmd(nc, in_maps, list(range(N_CORES)), trace=trace)
    kernel._last_exec_ns = res.exec_time_ns
    kernel._last_mean_exec_ns = res.mean_exec_time_ns

    out = np.zeros((B, S, D), np.float64)
    for c in range(N_CORES):
        out[c // 4] += res.results[c]["outp"].astype(np.float64)
    out += bo.astype(np.float64)
    return out.astype(np.float32)


# revision 72
# speedup vs baseline: 1.0013x; 1.0013x over previous
"""Multi-head attention (B=2, S=2048, D=1024, H=16, causal) on 8 Trainium2
NeuronCores via Bass/Tile.

Sharding: core c -> batch c//4, heads [4*(c%4), 4*(c%4)+4)  (DP over batch x
TP over heads).  QKV weights column-parallel, O row-parallel; the 4 partial
[S, D] outputs per batch are summed on the host (gather step), bias bo added
there too.

v2 dataflow (bf16 matmuls, fp32 PSUM accumulation), per core:
  - single sync-engine DMA queue in needed-by order: wk, bk, xk tiles woven
    with wq/cm/..., so the first K-proj matmul isn't stuck behind weights.
  - K proj -> kt [128 (2 heads x 64dk), S] x2 pairs; Q proj -> qt same.
  - scores for qc=0 are woven into the Q/V projection phase (Scalar exp
    starts ~25us in instead of ~55us).
  - scores computed transposed per kc block [128 k, 2, 512 q] covering BOTH
    heads of a pair; the two matmuls use partition rows 0:64 / 64:128 so the
    PE row-group tiling runs them concurrently.  ONE exp ACT covers both
    heads.  Diagonal blocks restrict to the causally valid q range
    (cols >= 128*j) for scores, exp, and attnV; the 128-wide diagonal strip
    gets the triangular keep-mask on DVE.
  - attnV: po[65, 512] per head; row 64 accumulates the softmax denominator
    via the ones column in V'.  kc loop is software-pipelined (attnV lags
    scores by 2) so the PE never waits on the exp chain.
  - normalize: DVE reciprocal of the denominator row -> rcpg[4, 512] bf16;
    one PE matmul (sel2) broadcasts a PAIR's two reciprocal rows to
    [128, 512]; gpsimd multiplies into outT2[pair] [128 (2 heads), S] bf16.
  - O proj: out[q, D] = sum_p outT2[p].T @ wo2[p] -- 2 accumulating matmuls
    per [128, 512] tile (full 128-deep contraction).
  - bc/normalize/O-proj work is queued as "fillers" popped one per kc
    iteration so the PE stream has no micro-idles (HAM stays at K=8/8).
"""

import os
import sys
import types
from collections import deque

import numpy as np

B, S, D, H = 2, 2048, 1024, 16
DK = D // H  # 64
N_CORES = 8
HPC = 4  # heads per core
SCALE = 1.0 / np.sqrt(np.float32(DK))  # folded into Wq/bq on host

QC = 512  # query block (free dim of scores matmuls)
KC = 128  # key block (partition dim of transposed scores)
NQC = S // QC  # 4
LAG = 4  # attnV trails scores by LAG kc-iterations


def _install_ntff_hook():
    """The image's antenv lacks axon_hooks; register the NTFF profile hook
    ourselves so run_bass_kernel_spmd(trace=True) works."""
    if "antenv.axon_hooks" in sys.modules:
        return
    try:
        mod = types.ModuleType("antenv.axon_hooks")
        state = {"hook": None}
        mod.set_axon_ntff_profile_hook = lambda h: state.__setitem__("hook", h)
        mod.get_axon_ntff_profile_hook = lambda: state["hook"]
        sys.modules["antenv.axon_hooks"] = mod
        from trn_agent_boot.trn_boot import _ntff_profile_via_ctypes

        mod.set_axon_ntff_profile_hook(
            _ntff_profile_via_ctypes("/opt/axon/libaxon_pjrt.so")
        )
    except Exception:
        sys.modules.pop("antenv.axon_hooks", None)


def _split_multi_waits(nc):
    """This walrus build accepts at most ONE sem wait per instruction; Tile
    packs several.  Split extras into preceding single-wait NOPs on the same
    engine (equivalent semantics: the engine blocks on them in order)."""
    import bass_rust

    cnt = 0
    for bbw in nc.main_func.blocks:
        bb = bbw.bb if hasattr(bbw, "bb") else bbw
        out = []
        changed = False
        for ins in bb.instructions:
            si = ins.sync_info
            if si is not None and len(si.on_wait) > 1:
                changed = True
                waits = list(si.on_wait)
                for w in waits[:-1]:
                    cnt += 1
                    nop = bass_rust.InstNoOp(name=f"I-wsp{cnt}", ins=[], outs=[])
                    nop.engine = ins.engine
                    nop.sync_info = bass_rust.SyncInfo(on_wait=[w], on_update=[])
                    out.append(nop)
                si.on_wait = [waits[-1]]
                ins.sync_info = si
            out.append(ins)
        if changed:
            bb.instructions = out
    return cnt


def _build_nc(split=True):
    from contextlib import ExitStack

    import concourse.bass as bass
    import concourse.tile as tile
    from concourse import mybir

    bf16 = mybir.dt.bfloat16
    f32 = mybir.dt.float32

    nc = bass.Bass()
    xqT = nc.declare_dram_parameter("xqT", [D, S], bf16, isOutput=False)
    xkT = nc.declare_dram_parameter("xkT", [D, S], bf16, isOutput=False)
    xvT = nc.declare_dram_parameter("xvT", [D, S], bf16, isOutput=False)
    wq = nc.declare_dram_parameter("wq", [128, 8 * 256], bf16, isOutput=False)
    wk = nc.declare_dram_parameter("wk", [128, 8 * 256], bf16, isOutput=False)
    wv = nc.declare_dram_parameter("wv", [128, 8 * 260], bf16, isOutput=False)
    wo2 = nc.declare_dram_parameter("wo2", [128, 2048], bf16, isOutput=False)
    bq = nc.declare_dram_parameter("bq", [128, 2], f32, isOutput=False)
    bk = nc.declare_dram_parameter("bk", [128, 2], f32, isOutput=False)
    bvp = nc.declare_dram_parameter("bvp", [1, 260], f32, isOutput=False)
    cm2 = nc.declare_dram_parameter("cm2", [128, 256], bf16, isOutput=False)
    sel2 = nc.declare_dram_parameter("sel2", [128, 256], bf16, isOutput=False)
    outp = nc.declare_dram_parameter("outp", [S, D], bf16, isOutput=True)

    with tile.TileContext(nc) as tc, ExitStack() as ctx:
        consts = ctx.enter_context(tc.tile_pool(name="consts", bufs=1))
        xs = ctx.enter_context(tc.tile_pool(name="xs", bufs=24))
        acts = ctx.enter_context(tc.tile_pool(name="acts", bufs=1))
        exps = ctx.enter_context(tc.tile_pool(name="exps", bufs=26))
        posbp = ctx.enter_context(tc.tile_pool(name="posbp", bufs=4))
        scrp = ctx.enter_context(tc.tile_pool(name="scrp", bufs=4))
        bcsp = ctx.enter_context(tc.tile_pool(name="bcsp", bufs=2))
        osb = ctx.enter_context(tc.tile_pool(name="osb", bufs=4))
        ps_sc = ctx.enter_context(tc.tile_pool(name="ps_sc", bufs=2, space="PSUM"))
        ps1 = ctx.enter_context(tc.tile_pool(name="ps1", bufs=2, space="PSUM"))

        # ---- persistent activation tiles ----
        kt = [acts.tile([128, S], bf16, name=f"kt{m}", tag=f"kt{m}") for m in range(2)]
        qt = [acts.tile([128, S], bf16, name=f"qt{m}", tag=f"qt{m}") for m in range(2)]
        vh_sb = acts.tile([128, 16, 260], bf16, name="vh", tag="vh")
        outT2 = [
            acts.tile([128, S], bf16, name=f"outT2_{p}", tag=f"outT2_{p}")
            for p in range(2)
        ]
        # reciprocal rows live at partitions 32*(2*pair+hh); other partitions
        # stay at the memset value so the sel2 matmul contracts over zeros,
        # never garbage (0*NaN would poison the broadcast).
        rcpg = acts.tile([128, 512], bf16, name="rcpg", tag="rcpg")

        # ---- DMA issue, needed-by order, all on the sync-engine HW queue ----
        # wk comes as two single-writer half tiles so the first projection
        # matmuls only wait on the 256KB they actually read
        wk_h = [consts.tile([128, 1024], bf16, name=f"wk_h{i}") for i in range(2)]
        nc.sync.dma_start(out=wk_h[0][:], in_=wk[:, 0:1024])

        xt = {}  # (input, dc, half) -> [128, 1024] bf16 tile

        def dma_x(src_, key, dc, half):
            t = xs.tile([128, S // 2], bf16, name="xt", tag="xt")
            nc.sync.dma_start(
                out=t[:],
                in_=src_[dc * 128:(dc + 1) * 128, half * 1024:(half + 1) * 1024],
            )
            xt[(key, dc, half)] = t

        dma_x(xkT, "k", 0, 0)
        nc.sync.dma_start(out=wk_h[1][:], in_=wk[:, 1024:2048])
        bk_sb = consts.tile([128, 2], f32, name="bk_sb")
        nc.sync.dma_start(out=bk_sb[:], in_=bk[:])
        for dc in range(1, 8):
            dma_x(xkT, "k", dc, 0)
        wq_sb = consts.tile([128, 8 * 256], bf16, name="wq_sb")
        nc.sync.dma_start(out=wq_sb[:], in_=wq[:])
        bq_sb = consts.tile([128, 2], f32, name="bq_sb")
        nc.sync.dma_start(out=bq_sb[:], in_=bq[:])
        for dc in range(8):
            dma_x(xkT, "k", dc, 1)
        cm2_sb = consts.tile([128, 2, 128], bf16, name="cm2_sb")
        nc.sync.dma_start(out=cm2_sb[:], in_=cm2[:])
        sel2_sb = consts.tile([128, 256], bf16, name="sel2_sb")
        nc.sync.dma_start(out=sel2_sb[:], in_=sel2[:])
        nc.vector.memset(rcpg[:], 0.0)
        ones1 = consts.tile([1, 512], f32, name="ones1")
        nc.vector.memset(ones1[:], 1.0)
        bvp_sb = consts.tile([128, 260], f32, name="bvp_sb")
        nc.sync.dma_start(out=bvp_sb[:], in_=bvp[:].to_broadcast((128, 260)))
        for half in range(2):
            for dc in range(8):
                dma_x(xqT, "q", dc, half)
        wv_sb = consts.tile([128, 8 * 260], bf16, name="wv_sb")
        nc.sync.dma_start(out=wv_sb[:], in_=wv[:])
        for half in range(2):
            for dc in range(8):
                dma_x(xvT, "v", dc, half)
        wo2_sb = consts.tile([128, 2048], bf16, name="wo2_sb")
        nc.sync.dma_start(out=wo2_sb[:], in_=wo2[:])

        # ---- projection helpers ----
        # dc is the OUTER loop so the first matmul only waits on the first
        # input tile (the PE paces with the DMA stream instead of stalling
        # for all 8 chunks).
        def kq_sc(key, wchunk, bsb, dst, sc):
            ps = [ps1.tile([128, 512], f32, name="ps", tag="ps") for _ in range(2)]
            for dc in range(8):
                wt, base = wchunk(dc)
                for m in range(2):
                    nc.tensor.matmul(
                        ps[m][:],
                        lhsT=wt[:, base + m * 128: base + (m + 1) * 128],
                        rhs=xt[(key, dc, sc // 2)][
                            :, (sc % 2) * 512:(sc % 2) * 512 + 512
                        ],
                        start=(dc == 0),
                        stop=(dc == 7),
                    )
            for m in range(2):
                nc.vector.tensor_scalar_add(
                    dst[m][:, sc * 512:(sc + 1) * 512], ps[m][:], bsb[:, m:m + 1]
                )

        def v_stpair(sp):
            sts = (2 * sp, 2 * sp + 1)
            ps = [ps1.tile([128, 512], f32, name="ps", tag="ps") for _ in range(2)]
            for dc in range(8):
                for i, st in enumerate(sts):
                    nc.tensor.matmul(
                        ps[i][:, :260],
                        lhsT=xt[("v", dc, st // 8)][
                            :, (st % 8) * 128:(st % 8 + 1) * 128
                        ],
                        rhs=wv_sb[:, dc * 260:(dc + 1) * 260],
                        start=(dc == 0),
                        stop=(dc == 7),
                    )
            for i, st in enumerate(sts):
                nc.vector.tensor_add(vh_sb[:, st, :], ps[i][:, :260], bvp_sb[:])

        # ---- attention helpers ----
        def emit_scores(qc, kc, pair):
            """scores + exp (+ causal mask) for one kc block, both heads of
            the pair.  Returns (ex tile, lo) for the matching attnV."""
            j = kc - 4 * qc  # diagonal sub-block index, or negative
            lo = 128 * j if j >= 0 else 0
            pss = ps_sc.tile([128, 2, 512], f32, name="pss", tag="pss")
            for hh in range(2):
                hr = slice(64 * hh, 64 * hh + 64)
                nc.tensor.matmul(
                    pss[:, hh, lo:],
                    lhsT=kt[pair][hr, kc * 128:(kc + 1) * 128],
                    rhs=qt[pair][hr, qc * QC + lo:(qc + 1) * QC],
                    start=True,
                    stop=True,
                )
            ex = exps.tile([128, 2, 512], bf16, name="ex", tag="ex")
            nc.scalar.activation(
                ex[:, :, lo:], pss[:, :, lo:], mybir.ActivationFunctionType.Exp
            )
            if j >= 0:
                # triangular keep-mask on the 128-wide diagonal strip
                nc.vector.tensor_mul(
                    ex[:, :, lo:lo + 128], ex[:, :, lo:lo + 128], cm2_sb[:]
                )
            return ex, lo

        def emit_attnv(qc, kc, pair, po, ex, lo):
            last = 4 * qc + 3
            for hh in range(2):
                h = 2 * pair + hh
                nc.tensor.matmul(
                    po[hh][:, lo:],
                    lhsT=vh_sb[:, kc, h * 65:(h + 1) * 65],
                    rhs=ex[:, hh, lo:],
                    start=(kc == 0),
                    stop=(kc == last),
                )

        def pair_end(qc, pair, po):
            """free the po PSUM banks fast: po[hh] frees after one Ln (ACT)
            and one staging copy (DVE), running in parallel.  The Exp half of
            the reciprocal is deferred past the next pair's first exps; bc2
            only needs rcpg ~3 kc later."""
            posb2 = posbp.tile([128, 512], bf16, name="posb2", tag="posb2")
            lgs = []
            with nc.allow_low_precision(reason="attn-out staged bf16"):
                for hh in range(2):
                    lg = scrp.tile([1, 512], f32, name="lg", tag="lg")
                    nc.scalar.activation(
                        lg[:], po[hh][64:65, :], mybir.ActivationFunctionType.Ln
                    )
                    nc.vector.tensor_copy(
                        posb2[64 * hh:64 * hh + 64, :], po[hh][0:64, :]
                    )
                    lgs.append(lg)

            def emit_lns():
                pass

            def emit_rcp():
                with nc.allow_low_precision(reason="rcp bf16, as baseline"):
                    for hh in range(2):
                        r = 32 * (2 * pair + hh)
                        nc.scalar.activation(
                            rcpg[r:r + 1, :],
                            lgs[hh][:],
                            mybir.ActivationFunctionType.Exp,
                            scale=-1.0,
                        )
            return posb2, emit_lns, emit_rcp

        def make_bc_norm(qc, pair, posb2):
            def emit():
                bcps = ps1.tile([128, 512], f32, name="ps", tag="ps")
                nc.tensor.matmul(
                    bcps[:],
                    lhsT=sel2_sb[:, pair * 128:(pair + 1) * 128],
                    rhs=rcpg[:],
                    start=True,
                    stop=True,
                )
                bcs2 = bcsp.tile([128, 512], bf16, name="bcs2", tag="bcs2")
                nc.vector.tensor_copy(bcs2[:], bcps[:])
                nc.gpsimd.tensor_mul(
                    outT2[pair][:, qc * QC:(qc + 1) * QC], posb2[:], bcs2[:]
                )
            return emit

        def make_oproj(qc, g):
            def emit():
                sti, ns = g // 2, g % 2
                st = qc * 4 + sti
                ps = ps1.tile([128, 512], f32, name="ps", tag="ps")
                for p in range(2):
                    nc.tensor.matmul(
                        ps[:],
                        lhsT=outT2[p][:, st * 128:(st + 1) * 128],
                        rhs=wo2_sb[:, p * 1024 + ns * 512: p * 1024 + ns * 512 + 512],
                        start=(p == 0),
                        stop=(p == 1),
                    )
                ot = osb.tile([128, 512], bf16, name="ot", tag="ot")
                with nc.allow_low_precision(reason="partial out bf16, host f64 sum"):
                    nc.vector.tensor_copy(ot[:], ps[:])
                nc.sync.dma_start(
                    out=outp[st * 128:(st + 1) * 128, ns * 512:(ns + 1) * 512],
                    in_=ot[:],
                )
            return emit

        # ---- projections, with qc=0 and qc=1 scores woven in ----
        wk_chunk = lambda dc: (wk_h[dc // 4], (dc % 4) * 256)
        wq_chunk = lambda dc: (wq_sb, dc * 256)
        pre = {}  # (qc, pair, kc) -> (ex, lo)
        for sc in range(4):
            kq_sc("k", wk_chunk, bk_sb, kt, sc)
        kq_sc("q", wq_chunk, bq_sb, qt, 0)
        for sc in range(1, 4):
            for pair in range(2):
                pre[(0, pair, sc - 1)] = emit_scores(0, sc - 1, pair)
            kq_sc("q", wq_chunk, bq_sb, qt, sc)
        for pair in range(2):
            pre[(0, pair, 3)] = emit_scores(0, 3, pair)
        for sp in range(8):
            v_stpair(sp)
            for pair in range(2):
                pre[(1, pair, sp)] = emit_scores(1, sp, pair)

        # ---- attention main loop ----
        # fillers: (cond, emit_fn) -- PE work from earlier qc woven into the
        # kc loop.  cond(kc) delays items whose dependency chain (rcp -> bc2
        # -> norm) is still settling so the PE doesn't stall on them, and
        # gates oproj(qc) on its norm having been EMITTED (program order).
        fillers = deque()
        norm_done = set()  # (qc, pair) whose bc_norm has been emitted
        rcp_done = set()  # (qc, pair) whose deferred Exp has been emitted
        pending_lns = []  # deferred Ln emitters from the previous pair_end
        pending_exps = []  # deferred Exp emitters from the previous pair_end

        def pop_filler(kc):
            for i, (kind, cond, fn) in enumerate(fillers):
                if cond(kc):
                    del fillers[i]
                    fn()
                    return

        for qc in range(NQC):
            for pair in range(2):
                po = [
                    ps1.tile([65, 512], f32, name=f"po{hh}", tag="po")
                    for hh in range(2)
                ]
                nkc = 4 * qc + 4
                if (qc, pair, 0) in pre:
                    for kc in range(nkc):
                        if kc == 0:
                            for fn in pending_lns:
                                fn()
                            pending_lns.clear()
                        ex, lo = pre.pop((qc, pair, kc))
                        emit_attnv(qc, kc, pair, po, ex, lo)
                        if kc == 1:
                            for fn in pending_exps:
                                fn()
                            pending_exps.clear()
                        if kc >= 1:
                            pop_filler(kc)
                else:
                    meta = {}
                    for kc in range(nkc + LAG):
                        if kc < nkc:
                            meta[kc] = emit_scores(qc, kc, pair)
                        if kc == 0:
                            for fn in pending_lns:
                                fn()
                            pending_lns.clear()
                        if kc == 1:
                            for fn in pending_exps:
                                fn()
                            pending_exps.clear()
                        if kc >= LAG:
                            ex, lo = meta.pop(kc - LAG)
                            emit_attnv(qc, kc - LAG, pair, po, ex, lo)
                        # every-other-kc pops leave fillers in reserve for the
                        # pair boundary unless the backlog is deep
                        if kc >= 1 and (kc % 2 == 1 or len(fillers) > 6):
                            pop_filler(kc)
                posb2, emit_lns, emit_rcp = pair_end(qc, pair, po)
                pending_lns.append(emit_lns)

                def rcp_wrap(qc=qc, pair=pair, emit_rcp=emit_rcp):
                    emit_rcp()
                    rcp_done.add((qc, pair))

                pending_exps.append(rcp_wrap)
                inner = make_bc_norm(qc, pair, posb2)

                def bc_wrap(qc=qc, pair=pair, inner=inner):
                    inner()
                    norm_done.add((qc, pair))

                fillers.append(
                    (
                        "bc",
                        lambda kc, qc=qc, pair=pair: kc >= 3
                        and (qc, pair) in rcp_done,
                        bc_wrap,
                    )
                )
                if pair == 1:
                    for g in range(8):
                        fillers.append(
                            (
                                "op",
                                lambda kc, qc=qc: (qc, 1) in norm_done,
                                make_oproj(qc, g),
                            )
                        )
                # boundary: give the PE ready work (kc-independent fillers)
                # while the po-free / exp chains settle
                pop_filler(-1)
                pop_filler(-1)
                pop_filler(-1)
        for fn in pending_lns:
            fn()
        pending_lns.clear()
        for fn in pending_exps:
            fn()
        pending_exps.clear()
        while fillers:
            fillers.popleft()[2]()

    if split:
        _split_multi_waits(nc)
    return nc


_NC_CACHE = None


def _get_nc():
    global _NC_CACHE
    if _NC_CACHE is None:
        _NC_CACHE = _build_nc()
    return _NC_CACHE


def _swizzle_w(wT, block):
    """wT [D, C] -> [128, 8*C] so that out[p, dc*C + j] = wT[dc*128 + p, j]."""
    dcs = wT.shape[0] // 128
    return np.ascontiguousarray(
        wT.reshape(dcs, 128, wT.shape[1]).transpose(1, 0, 2).reshape(128, -1)
    )


def _np_reference(q, k, v, mask, Wq, bq, Wk, bk, Wv, bv, Wo, bo):
    def split_heads(x):
        b, s, _ = x.shape
        return x.reshape(b, s, H, DK).transpose(0, 2, 1, 3)

    qh = split_heads(q @ Wq.T + bq)
    kh = split_heads(k @ Wk.T + bk)
    vh = split_heads(v @ Wv.T + bv)
    scores = np.einsum("bhqd,bhkd->bhqk", qh, kh) / np.sqrt(np.float32(DK))
    scores = np.where(mask, np.float32(-1e9), scores)
    scores = scores - scores.max(axis=-1, keepdims=True)
    e = np.exp(scores)
    attn = e / e.sum(axis=-1, keepdims=True)
    out = np.einsum("bhqk,bhkd->bhqd", attn, vh)
    out = out.transpose(0, 2, 1, 3).reshape(q.shape[0], -1, D)
    return (out @ Wo.T + bo).astype(np.float32)


def kernel(q, k, v, mask, Wq, bq, Wk, bk, Wv, bv, Wo, bo):
    import ml_dtypes

    bf16 = ml_dtypes.bfloat16

    q = np.asarray(q, np.float32)
    k = np.asarray(k, np.float32)
    v = np.asarray(v, np.float32)
    mask = np.asarray(mask, bool)
    Wq = np.asarray(Wq, np.float32)
    bq = np.asarray(bq, np.float32)
    Wk = np.asarray(Wk, np.float32)
    bk = np.asarray(bk, np.float32)
    Wv = np.asarray(Wv, np.float32)
    bv = np.asarray(bv, np.float32)
    Wo = np.asarray(Wo, np.float32)
    bo = np.asarray(bo, np.float32)

    causal = np.triu(np.ones((S, S), dtype=bool), k=1)
    if not np.array_equal(mask.reshape(S, S), causal):
        return _np_reference(q, k, v, mask, Wq, bq, Wk, bk, Wv, bv, Wo, bo)

    _install_ntff_hook()
    from concourse.bass_utils import run_bass_kernel_spmd

    nc = _get_nc()

    # triangular keep-mask for the 128-wide diagonal strip, doubled for the
    # two heads sharing one exp tile: keep iff kk <= qq
    kk = np.arange(128)[:, None]
    qq = np.arange(128)[None, :]
    tri = (kk <= qq).astype(bf16)
    cm2_np = np.concatenate([tri, tri], axis=1)  # [128, 256]

    # sel2[32*(2p+hh), p*128 + m] = 1 for m in the hh half: broadcast-select
    # the pair's two reciprocal rows (at partitions 0/32/64/96) onto 128
    sel2_np = np.zeros((128, 256), np.float32)
    for p in range(2):
        sel2_np[32 * (2 * p), p * 128:p * 128 + 64] = 1.0
        sel2_np[32 * (2 * p + 1), p * 128 + 64:p * 128 + 128] = 1.0
    sel2_np = sel2_np.astype(bf16)

    xT = {}
    for name, x in (("q", q), ("k", k), ("v", v)):
        xT[name] = [np.ascontiguousarray(x[b].T).astype(bf16) for b in range(B)]

    in_maps = []
    for c in range(N_CORES):
        b = c // 4
        g = c % 4
        hs = slice(g * HPC * DK, (g + 1) * HPC * DK)  # 256 rows of W, cols of Wo
        wq_c = _swizzle_w((SCALE * Wq[hs]).T.astype(bf16), 256)
        wk_c = _swizzle_w(Wk[hs].T.astype(bf16), 256)
        # V' with a zero weight column at h*65+64 (ones come via bias row)
        wvT = Wv[hs].T  # [1024, 256]
        wvp = np.zeros((D, 260), np.float32)
        for h in range(HPC):
            wvp[:, h * 65:h * 65 + 64] = wvT[:, h * 64:(h + 1) * 64]
        wv_c = _swizzle_w(wvp.astype(bf16), 260)
        # wo2: pair p columns hold (Wo[:, hs].T)[p*128:(p+1)*128, :]
        woT = np.ascontiguousarray(Wo[:, hs].T)  # [256, 1024]
        wo2_c = np.concatenate([woT[0:128], woT[128:256]], axis=1).astype(bf16)
        bq_c = np.ascontiguousarray(
            (SCALE * bq[hs]).reshape(2, 128).T.astype(np.float32)
        )
        bk_c = np.ascontiguousarray(bk[hs].reshape(2, 128).T.astype(np.float32))
        bvp_c = np.zeros((1, 260), np.float32)
        for h in range(HPC):
            bvp_c[0, h * 65:h * 65 + 64] = bv[hs][h * 64:(h + 1) * 64]
            bvp_c[0, h * 65 + 64] = 1.0
        in_maps.append(
            {
                "xqT": xT["q"][b],
                "xkT": xT["k"][b],
                "xvT": xT["v"][b],
                "wq": wq_c,
                "wk": wk_c,
                "wv": wv_c,
                "wo2": wo2_c,
                "bq": bq_c,
                "bk": bk_c,
                "bvp": bvp_c,
                "cm2": cm2_np,
                "sel2": sel2_np,
            }
        )

    trace = bool(os.environ.get("BASSMHA_TRACE"))
    res = run_bass_kernel_spmd(nc, in_maps, list(range(N_CORES)), trace=trace)
    kernel._last_exec_ns = res.exec_time_ns
    kernel._last_mean_exec_ns = res.mean_exec_time_ns

    out = np.zeros((B, S, D), np.float64)
    for c in range(N_CORES):
        out[c // 4] += res.results[c]["outp"].astype(np.float64)
    out += bo.astype(np.float64)
    return out.astype(np.float32)
